# revision 18
# baseline (speedup 1.0000x reference)
"""Trainium2 Bass kernel for nn_Encoder (dense transformer encoder layer).

Sharding: 8 NeuronCores, sequence-parallel, zero collectives. B*S = 4096
rows -> 512 rows per core; cores 0-3 handle batch 0, cores 4-7 batch 1.
The full batch's x (transposed, fp8e4) is replicated to every core as an
input, so each core computes K^T and V for the WHOLE batch locally (no
K/V AllGather), then runs attention for its own 512 query rows over all
16 heads, plus Wo / LN1 / FFN / LN2 locally.

FP8 (e4m3) DoubleRow matmuls for the big-contraction GEMMs: K/Q/V
projections, P@V, FFN1, FFN2 (2 fp8 values per PE cell -> 2x contraction
per cycle). Weights are pre-scaled x32 (W2 x64) host-side so they sit in
e4m3 normal range; the inverse scales are folded into the PSUM
evacuations (tensor_scalar mult+add / activation scale). Scores matmul
stays bf16 (output-rate-bound at K=64, fp8 gains nothing); Wo stays
bf16. Residual path keeps bf16 precision. PSUM always fp32.

x is shipped pre-transposed in a piece-major layout [128, 4, 8, 512]
(p, key-piece, dc, col) so each of the 4 input DMAs is one contiguous
4KB line per partition AND matmul APs can slice (dc, dc+1) pairs for
DoubleRow directly. First projection matmul starts ~2us in.

Schedule: every projection chunk (K/Q/V) carries a (pair, unit)
deadline; chunks are emitted just-in-time between attention units.

Dataflow per core:
  xTs [128, 4, 8, 512] fp8 (host-transposed full-batch input)
  kT[p] [128, 2048] bf16 = (Wk_p'.T @ xT)/32 + bk  (full batch keys)
  vq[half]: V' = 32*V + 32*bv columns (fp8) for 4 pairs, keys on
            partition, ones column at col 64, pad cols 65:80 zero
  qT[p] [128, 512] bf16 from xoT fp8 (own rows)
  S^T [128 keys, 512 q] = kts.T @ qT  (2 heads packed at rows 0/64)
  P = exp(0.125 * S^T) -> fp8, one ACT op per 2 key tiles
  O^T[80,512] += DoubleRow([V'|1|0].T @ P)  (psum row 64 = denominator)
  oT = O'^T * bcast((1/32)*(1/den));  x1 = sum_dc oT_chunk.T @ Wo_rows
       + (x + bo)  [bo folded host-side into xn]
  LN1 -> x1n bf16 -> x1nT fp8;  hT = relu((W1'.T @ x1nT)/32 + b1) fp8
  ffn = (sum_f DoubleRow(hT_chunk.T @ W2'_rows))/64; x2 = ffn + x1n + b2
  LN2 -> out [512, 1024] f32
"""

from collections import deque

import numpy as np

import concourse.bass as bass
import concourse.mybir as mybir
from concourse import bacc
from concourse.tile import TileContext
from concourse.bass_utils import run_bass_kernel_spmd

F32 = mybir.dt.float32
BF = mybir.dt.bfloat16
F8 = mybir.dt.float8e4
AF = mybir.ActivationFunctionType
OP = mybir.AluOpType
DR = mybir.MatmulPerfMode.DoubleRow

B, S, D = 2, 2048, 1024
H, DK, DFF = 16, 64, 4096
NCORES = 8
RPC = S * B // NCORES          # 512 own rows per core
FB = S                         # 2048 full-batch rows
NP = H // 2                    # 8 head pairs
NKT = FB // 128                # 16 key tiles
VC = 80                        # padded V column block (64 V + 1 ones + 15 pad)
WS = 32.0                      # weight pre-scale for Wk/Wq/Wv/W1
WS2 = 64.0                     # weight pre-scale for W2

_TCNT = [0]


def _mk(pool, shape, dt, tag, bufs=None):
    _TCNT[0] += 1
    kw = {} if bufs is None else {"bufs": bufs}
    return pool.tile(shape, dt, tag=tag, name=f"t{_TCNT[0]}_{tag}", **kw)


def build_nc():
    nc = bacc.Bacc(num_devices=NCORES)

    ins = dict(
        xbT=nc.dram_tensor("xbT", [128, 4 * 8 * 512], F8, kind="ExternalInput"),
        xoT=nc.dram_tensor("xoT", [128, 8 * 512], F8, kind="ExternalInput"),
        xn=nc.dram_tensor("xn", [RPC, D], BF, kind="ExternalInput"),
        wkq=nc.dram_tensor("wkq", [NP, 128, 2048], F8, kind="ExternalInput"),
        wv=nc.dram_tensor("wv", [2, 128, 4096], F8, kind="ExternalInput"),
        wo=nc.dram_tensor("wo", [2, 128, 4096], BF, kind="ExternalInput"),
        w1=nc.dram_tensor("w1", [32, 128, 1024], F8, kind="ExternalInput"),
        w2=nc.dram_tensor("w2", [128, 32 * 1024], F8, kind="ExternalInput"),
        bq=nc.dram_tensor("bq", [D, 1], F32, kind="ExternalInput"),
        bk=nc.dram_tensor("bk", [D, 1], F32, kind="ExternalInput"),
        bvr=nc.dram_tensor("bvr", [1, D], BF, kind="ExternalInput"),
        b1=nc.dram_tensor("b1", [DFF, 1], F32, kind="ExternalInput"),
        b2r=nc.dram_tensor("b2r", [1, D], BF, kind="ExternalInput"),
        g1=nc.dram_tensor("g1", [1, D], BF, kind="ExternalInput"),
        be1=nc.dram_tensor("be1", [1, D], BF, kind="ExternalInput"),
        g2=nc.dram_tensor("g2", [1, D], BF, kind="ExternalInput"),
        be2=nc.dram_tensor("be2", [1, D], BF, kind="ExternalInput"),
        ident=nc.dram_tensor("ident", [128, 128], BF, kind="ExternalInput"),
        out=nc.dram_tensor("out", [RPC, D], F32, kind="ExternalOutput"),
    )

    with TileContext(nc) as tc:
        _body(nc, tc, ins)

    nc.finalize()
    return nc


def _body(nc, tc, ins):
    with (
        tc.tile_pool(name="outer", bufs=1) as po,
        tc.tile_pool(name="psum", bufs=1, space="PSUM") as pp,
    ):
        # ---- constants (bvr first: needed by the first V evacuation) ----
        bvr = _mk(po, [128, D], BF, "bvr")
        nc.gpsimd.dma_start(out=bvr[:], in_=ins["bvr"].broadcast_to([128, D]))
        ident = _mk(po, [128, 128], BF, "ident")
        nc.gpsimd.dma_start(out=ident[:], in_=ins["ident"][:])
        bias = {}
        for nm, n in (("bq", 8), ("bk", 8), ("b1", 32)):
            t = _mk(po, [128, n], F32, "b_" + nm)
            nc.gpsimd.dma_start(out=t[:],
                                in_=ins[nm].rearrange("(i p) o -> p (i o)", p=128))
            bias[nm] = t
        b2r = _mk(po, [128, D], BF, "b2r")
        nc.gpsimd.dma_start(out=b2r[:], in_=ins["b2r"].broadcast_to([128, D]))
        lnw = {}
        for nm in ("g1", "be1", "g2", "be2"):
            t = _mk(po, [128, D], BF, "ln_" + nm)
            nc.gpsimd.dma_start(out=t[:], in_=ins[nm].broadcast_to([128, D]))
            lnw[nm] = t
        eps = _mk(po, [128, 1], F32, "eps")
        nc.vector.memset(eps[:], 1e-5)
        lnw["eps"] = eps
        # broadcast source for 1/den: value 1/WS folds the V' = WS*V scale
        ones_f = _mk(po, [128, 64], F32, "ones_f")
        nc.vector.memset(ones_f[:], 1.0 / WS)
        # xn is DMA'd late (inside the weave) so it doesn't block the
        # startup-critical xoT/wkq transfers.
        xn = [_mk(po, [128, D], BF, f"xn{r}") for r in range(4)]

        # persistent post-phase activations
        x1n = [_mk(po, [128, D], BF, f"x1n{r}") for r in range(4)]
        x1nb = [_mk(po, [128, D], BF, f"x1nb{r}") for r in range(4)]
        x1nT = _mk(po, [128, 8 * RPC], F8, "x1nT")
        w2t = _mk(po, [128, 32 * 1024], F8, "w2t")

        with tc.tile_pool(name="attn", bufs=1) as pa:
            _attn_phase(nc, tc, ins, po, pa, pp, ident, bias, bvr, b2r,
                        xn, x1n, x1nb, x1nT, w2t, lnw, ones_f)

        with tc.tile_pool(name="post", bufs=1) as pf:
            _post_phase(nc, tc, ins, pf, pp, bias, lnw, x1n, x1nb, x1nT, w2t)


def _attn_phase(nc, tc, ins, po, pa, pp, ident, bias, bvr, b2r,
                xn, x1n, x1nb, x1nT, w2t, lnw, ones_f):
    oT = [_mk(pa, [128, RPC], BF, f"oT{p}") for p in range(NP)]
    kTs, qTs, vqs = {}, {}, {}

    with tc.tile_pool(name="proj", bufs=1) as px:
        # x^T in piece-major layout: [p, piece, dc, col] — each piece DMA is
        # one contiguous 4KB line per partition; matmul APs slice (dc,dc+1)
        # pairs for DoubleRow.
        # startup-critical transfers: pieces 0/2 on the sync queue now;
        # wkq0, xoT, pieces 1/3 go on the scalar queue inside dma_w(0) so
        # wkq0 is first in that queue's order.
        xTs = _mk(px, [128, 4 * 8 * 512], F8, "xTs")
        xTv = xTs[:].rearrange("p (k d c) -> p k d c", k=4, d=8)
        xbTv = ins["xbT"].rearrange("p (k d c) -> p k d c", k=4, d=8)
        xoTs = _mk(px, [128, 8 * 512], F8, "xoTs")
        xoTv = xoTs[:].rearrange("p (d c) -> p d c", d=8)
        nc.sync.dma_start(out=xTv[:, 0], in_=xbTv[:, 0])
        nc.sync.dma_start(out=xTv[:, 2], in_=xbTv[:, 2])

        def proj_items(p):
            """(pair, unit, thunk) items, sorted by deadline."""
            st = {}

            def dma_w():
                wkqt = _mk(px, [128, 2048], F8, "wkq", bufs=2)
                nc.scalar.dma_start(out=wkqt[:], in_=ins["wkq"][p])
                if p == 0:
                    nc.scalar.dma_start(
                        out=xoTv[:],
                        in_=ins["xoT"].rearrange("p (d c) -> p d c", d=8))
                    nc.scalar.dma_start(out=xTv[:, 1], in_=xbTv[:, 1])
                    nc.scalar.dma_start(out=xTv[:, 3], in_=xbTv[:, 3])
                kTs[p] = _mk(pa, [128, FB], BF, "kT", bufs=3)
                qTs[p] = _mk(pa, [128, RPC], BF, "qT", bufs=3)
                st["wkt"] = wkqt[:, 0:1024].rearrange("p (d c) -> p d c", d=8)
                st["wqt"] = wkqt[:, 1024:2048].rearrange("p (d c) -> p d c", d=8)

            def k_chunk(c):
                ps = _mk(pp, [128, 512], F32, "ps_p", bufs=2)
                for dc in range(0, 8, 2):
                    nc.tensor.matmul(ps[:], st["wkt"][:, dc:dc + 2, :],
                                     xTv[:, c, dc:dc + 2, :],
                                     start=(dc == 0), stop=(dc == 6),
                                     perf_mode=DR, skip_group_check=True)
                nc.vector.tensor_scalar(kTs[p][:, c * 512:(c + 1) * 512],
                                        ps[:], 1.0 / WS, bias["bk"][:, p:p + 1],
                                        OP.mult, OP.add)

            def q_chunk():
                ps = _mk(pp, [128, 512], F32, "ps_p", bufs=2)
                for dc in range(0, 8, 2):
                    nc.tensor.matmul(ps[:], st["wqt"][:, dc:dc + 2, :],
                                     xoTv[:, dc:dc + 2, :],
                                     start=(dc == 0), stop=(dc == 6),
                                     perf_mode=DR, skip_group_check=True)
                nc.vector.tensor_scalar(qTs[p][:], ps[:], 1.0 / WS,
                                        bias["bq"][:, p:p + 1],
                                        OP.mult, OP.add)

            items = [(p, 0, dma_w), (p, 0, lambda: k_chunk(0)), (p, 0, q_chunk)]
            for c in range(1, 4):
                items.append((p, 2 * c, lambda c=c: k_chunk(c)))

            if p % 4 == 0:
                hf = p // 4

                def dma_v():
                    wvt = _mk(px, [128, 4096], F8, "wvt", bufs=1)
                    nc.sync.dma_start(out=wvt[:], in_=ins["wv"][hf])
                    vq = _mk(pa, [128, 4 * 2 * NKT * VC], F8, "vq", bufs=2)
                    vqs[hf] = vq
                    vqv = vq[:].rearrange("k (i h t c) -> k i h t c",
                                          i=4, h=2, c=VC)
                    nc.vector.memset(vqv[:, :, :, :, 64:VC], 0.0)
                    nc.vector.memset(vqv[:, :, :, :, 64:65], 1.0)
                    st["wvt"] = wvt[:].rearrange("p (d c) -> p d c", d=8)

                def v_chunk(kt2):
                    vqv = vqs[hf][:].rearrange("k (i h t c) -> k i h t c",
                                               i=4, h=2, c=VC)
                    bvs = bvr[:, hf * 512:(hf + 1) * 512].rearrange(
                        "k (i h c) -> k i h c", i=4, c=64)
                    ps = _mk(pp, [128, 512], F32, "ps_p", bufs=2)
                    for dc in range(0, 8, 2):
                        nc.tensor.matmul(
                            ps[:],
                            xTv[:, kt2 // 4, dc:dc + 2,
                                (kt2 % 4) * 128:(kt2 % 4 + 1) * 128],
                            st["wvt"][:, dc:dc + 2, :],
                            start=(dc == 0), stop=(dc == 6),
                            perf_mode=DR, skip_group_check=True)
                    nc.vector.tensor_tensor(
                        vqv[:, :, :, kt2, 0:64],
                        ps[:].rearrange("k (i h c) -> k i h c", i=4, c=64),
                        bvs, OP.add)

                items.append((p, 0, dma_v))
                for kt2 in range(NKT):
                    items.append((p, kt2 // 2, lambda kt2=kt2: v_chunk(kt2)))

            items.sort(key=lambda it: (it[0], it[1]))
            return items

        def attn_unit(p, u, pso):
            kt, qt, vq = kTs[p], qTs[p], vqs[p // 4]
            pl = p % 4
            vqv = vq[:].rearrange("k (i h t c) -> k i h t c", i=4, h=2, c=VC)
            pss = [_mk(pp, [128, 1024], F32, "ps_s", bufs=2) for _ in range(2)]
            for i in range(2):
                ktile = 2 * u + i
                for hh in range(2):
                    nc.tensor.matmul(
                        pss[hh][:, i * 512:(i + 1) * 512],
                        kt[hh * 64:(hh + 1) * 64,
                           ktile * 128:(ktile + 1) * 128],
                        qt[hh * 64:(hh + 1) * 64, :],
                        start=True, stop=True, skip_group_check=True)
            for hh in range(2):
                pt = _mk(pa, [128, 1024], F8, "pt", bufs=2)
                nc.scalar.activation(pt[:], pss[hh][:], AF.Exp,
                                     bias=0.0, scale=0.125)
                nc.tensor.matmul(
                    pso[hh][0:VC, :],
                    vqv[:, pl, hh, 2 * u:2 * u + 2, :],
                    pt[:].rearrange("k (i q) -> k i q", i=2),
                    start=(u == 0), stop=(u == NKT // 2 - 1),
                    perf_mode=DR, skip_group_check=True)

        def attn_stash(p, pso):
            """Copy raw O and den off PSUM at pair end (frees the banks);
            normalization is deferred into the next pair's stream."""
            st = []
            den = _mk(pa, [128, 512], BF, "dens", bufs=2)
            for hh in range(2):
                o_raw = _mk(pa, [128, 512], BF, "o_raw", bufs=2)
                nc.scalar.activation(o_raw[0:64, :], pso[hh][0:64, :], AF.Copy)
                nc.vector.tensor_copy(den[64 * hh:64 * hh + 1, :],
                                      pso[hh][64:65, :])
                st.append(o_raw)
            st.append(den)
            return st

        def attn_normalize(p, st):
            # 1/den via exp(-ln(den)) on the scalar engine: vector reciprocal
            # has a ~2.2us fixed cost per op and stalls the evacuation queue.
            den = st[2]
            rden = _mk(pa, [128, 512], F32, "rden", bufs=1)
            lnd = _mk(pa, [128, 512], F32, "lnd", bufs=1)
            for hh in range(2):
                nc.scalar.activation(lnd[64 * hh:64 * hh + 1, :],
                                     den[64 * hh:64 * hh + 1, :], AF.Ln)
                nc.scalar.activation(rden[64 * hh:64 * hh + 1, :],
                                     lnd[64 * hh:64 * hh + 1, :], AF.Exp,
                                     bias=0.0, scale=-1.0)
            for hh in range(2):
                o_raw = st[hh]
                ps_b = _mk(pp, [128, 512], F32, "ps_p", bufs=2)
                nc.tensor.matmul(ps_b[0:64, :],
                                 ones_f[64 * hh:64 * hh + 1, 0:64],
                                 rden[64 * hh:64 * hh + 1, :],
                                 start=True, stop=True,
                                 skip_group_check=True)
                rb = _mk(pa, [128, 512], F32, "rb", bufs=1)
                nc.scalar.activation(rb[0:64, :], ps_b[0:64, :], AF.Copy)
                # bv already folded into V' (softmax rows sum to 1)
                if hh == 0:
                    nc.gpsimd.tensor_tensor(oT[p][0:64, :], o_raw[0:64, :],
                                            rb[0:64, :], OP.mult)
                else:
                    stage = _mk(pa, [128, 512], BF, "stage", bufs=1)
                    nc.gpsimd.tensor_tensor(stage[0:64, :], o_raw[0:64, :],
                                            rb[0:64, :], OP.mult)
                    nc.gpsimd.dma_start(out=oT[p][64:128, :],
                                        in_=stage[0:64, :])

        # ---- deadline-scheduled weave (proj runs up to 2 pairs ahead;
        # kT/qT tag bufs=3 require never enqueueing more than p+2) ----
        pending = deque()
        for pre in range(min(2, NP)):
            pending.extend(proj_items(pre))

        def drain(n):
            for _ in range(min(n, len(pending))):
                pending.popleft()[2]()

        def drain_due(p, u):
            while pending and (pending[0][0], pending[0][1]) <= (p, u):
                pending.popleft()[2]()

        wo_big = [_mk(pa, [128, 4096], BF, f"wot{g}") for g in range(2)]

        def wo_slice(dc, hf):
            return wo_big[dc // 4][:, (dc % 4) * 1024 + hf * 512:
                                   (dc % 4) * 1024 + (hf + 1) * 512]

        wo_parts = {}

        def wo_part(r):
            ps = [_mk(pp, [128, 512], F32, "ps_p", bufs=2) for _ in range(2)]
            for dc in range(6):
                for hf in range(2):
                    nc.tensor.matmul(ps[hf][:],
                                     oT[dc][:, r * 128:(r + 1) * 128],
                                     wo_slice(dc, hf),
                                     start=(dc == 0), stop=(dc == 5),
                                     skip_group_check=True)
            parts = []
            for hf in range(2):
                part = _mk(pa, [128, 512], BF, "wopart", bufs=8)
                nc.scalar.activation(part[:], ps[hf][:], AF.Copy)
                parts.append(part)
            wo_parts[r] = parts

        stash = None
        for p in range(NP):
            if p + 2 < NP:
                pending.extend(proj_items(p + 2))
            pso = [_mk(pp, [128, 512], F32, "ps_o", bufs=2) for _ in range(2)]
            for u in range(NKT // 2):
                drain_due(p, u)
                attn_unit(p, u, pso)
                if u == 3 and stash is not None:
                    attn_normalize(p - 1, stash)
                # late weight loads ride the scalar/gpsimd queues whose
                # instruction streams are gated by attention progress (EXP /
                # stage copies), so they can't crowd the startup window.
                if p == 2 and u == 0:
                    for r in range(4):
                        nc.gpsimd.dma_start(
                            out=xn[r][:],
                            in_=ins["xn"][r * 128:(r + 1) * 128, :])
                if p == 4 and u == 0:
                    for g in range(2):
                        nc.scalar.dma_start(out=wo_big[g][:], in_=ins["wo"][g])
                if p == 4 and u == 4:
                    nc.scalar.dma_start(out=w2t[:], in_=ins["w2"][:])
                if (p, u) in ((6, 4), (6, 6), (7, 1), (7, 3)):
                    wo_part({(6, 4): 0, (6, 6): 1, (7, 1): 2, (7, 3): 3}[(p, u)])
                drain(2)
            stash = attn_stash(p, pso)
        attn_normalize(NP - 1, stash)
        drain(len(pending))

    # ---- Wo combine: add the dc 6..7 contributions, then LN1 ----
    pst = [_mk(pp, [128, 2048], BF, "ps_s", bufs=2) for _ in range(2)]
    x1nTv = x1nT[:].rearrange("p (d c) -> p d c", d=8)
    for r in range(4):
        ps = [_mk(pp, [128, 512], F32, "ps_p", bufs=2) for _ in range(2)]
        for dc in range(6, 8):
            for hf in range(2):
                nc.tensor.matmul(ps[hf][:],
                                 oT[dc][:, r * 128:(r + 1) * 128],
                                 wo_slice(dc, hf),
                                 start=(dc == 6), stop=(dc == 7),
                                 skip_group_check=True)
        x1 = _mk(po, [128, D], BF, "x1", bufs=2)
        for hf in range(2):
            t = _mk(po, [128, 512], BF, "x1t", bufs=2)
            nc.vector.tensor_tensor(t[:], ps[hf][:], wo_parts[r][hf][:],
                                    OP.add)
            nc.gpsimd.tensor_tensor(x1[:, hf * 512:(hf + 1) * 512],
                                    t[:], xn[r][:, hf * 512:(hf + 1) * 512],
                                    OP.add)
        _layernorm(nc, po, x1n[r], x1, lnw["g1"], lnw["be1"], lnw["eps"],
                   ew=nc.gpsimd)
        nc.gpsimd.tensor_tensor(x1nb[r][:], x1n[r][:], b2r[:], OP.add)
        for dc in range(8):
            nc.tensor.transpose(
                pst[dc // 4][:, (dc % 4) * 512 + r * 128:
                             (dc % 4) * 512 + (r + 1) * 128],
                x1n[r][:, dc * 128:(dc + 1) * 128], ident[:])
    for dc in range(8):
        nc.scalar.activation(x1nTv[:, dc, :],
                             pst[dc // 4][:, (dc % 4) * 512:
                                          (dc % 4 + 1) * 512], AF.Copy)


def _post_phase(nc, tc, ins, pf, pp, bias, lnw, x1n, x1nb, x1nT, w2t):
    # ---- FFN1: hT[f] = relu((W1'.T @ x1nT)/WS + b1), fp8 out ----
    hT = _mk(pf, [128, 32 * RPC], F8, "hT")
    hTv = hT[:].rearrange("p (f c) -> p f c", f=32)
    x1nTv = x1nT[:].rearrange("p (d c) -> p d c", d=8)
    w2v = w2t[:].rearrange("p (f c) -> p f c", f=32)
    for f in range(32):
        w1t = _mk(pf, [128, 1024], F8, "w1t", bufs=4)
        nc.sync.dma_start(out=w1t[:], in_=ins["w1"][f])
        w1v = w1t[:].rearrange("p (d c) -> p d c", d=8)
        ps = _mk(pp, [128, 512], F32, "ps_p", bufs=2)
        for dc in range(0, 8, 2):
            nc.tensor.matmul(ps[:], w1v[:, dc:dc + 2, :],
                             x1nTv[:, dc:dc + 2, :],
                             start=(dc == 0), stop=(dc == 6),
                             perf_mode=DR, skip_group_check=True)
        nc.scalar.activation(hTv[:, f, :], ps[:], AF.Relu,
                             bias=bias["b1"][:, f:f + 1], scale=1.0 / WS)

    # ---- FFN2 (DoubleRow) + residual + LN2 + out ----
    for r in range(4):
        ps = _mk(pp, [128, 1024], F32, "ps_s", bufs=2)
        for f in range(0, 32, 2):
            for hf in range(2):
                nc.tensor.matmul(ps[:, hf * 512:(hf + 1) * 512],
                                 hTv[:, f:f + 2, r * 128:(r + 1) * 128],
                                 w2v[:, f:f + 2, hf * 512:(hf + 1) * 512],
                                 start=(f == 0), stop=(f == 30),
                                 perf_mode=DR, skip_group_check=True)
        x2 = _mk(pf, [128, D], BF, "x2", bufs=2)
        for hf in range(2):
            sc = _mk(pf, [128, 512], BF, "ffnsc", bufs=2)
            nc.scalar.activation(sc[:], ps[:, hf * 512:(hf + 1) * 512],
                                 AF.Copy, bias=0.0, scale=1.0 / WS2)
            nc.vector.tensor_tensor(x2[:, hf * 512:(hf + 1) * 512], sc[:],
                                    x1nb[r][:, hf * 512:(hf + 1) * 512],
                                    OP.add)
        outt = _mk(pf, [128, D], F32, "outt", bufs=2)
        _layernorm(nc, pf, outt, x2, lnw["g2"], lnw["be2"], lnw["eps"])
        nc.sync.dma_start(out=ins["out"][r * 128:(r + 1) * 128, :],
                          in_=outt[:])


def _layernorm(nc, pool, out, x, g, be, eps, ew=None):
    """LN along the free dim (D=1024). x [128, 1024] bf16; out bf16/f32.

    1/std via exp(-0.5*ln(var+eps)) on the scalar engine (vector
    reciprocal has a ~2.2us fixed cost). `ew` picks the engine for the
    elementwise tail (default vector; gpsimd to offload busy regions).
    """
    if ew is None:
        ew = nc.vector
    _TCNT[0] += 1
    n = _TCNT[0]
    stats = pool.tile([128, 2, 6], F32, tag="ln_st", bufs=2, name=f"lnst{n}")
    for i in range(2):
        nc.vector.bn_stats(stats[:, i, :], x[:, i * 512:(i + 1) * 512])
    mv = pool.tile([128, 2], F32, tag="ln_mv", bufs=2, name=f"lnmv{n}")
    nc.vector.bn_aggr(mv[:], stats[:])
    lnv = pool.tile([128, 1], F32, tag="ln_sd", bufs=2, name=f"lnsd{n}")
    nc.scalar.activation(lnv[:], mv[:, 1:2], AF.Ln, bias=eps, scale=1.0)
    rstd = pool.tile([128, 1], F32, tag="ln_rs", bufs=2, name=f"lnrs{n}")
    nc.scalar.activation(rstd[:], lnv[:], AF.Exp, bias=0.0, scale=-0.5)
    t = pool.tile([128, D], BF, tag="ln_t", bufs=2, name=f"lnt{n}")
    ew.tensor_scalar(t[:], x[:], mv[:, 0:1], rstd[:],
                     OP.subtract, OP.mult)
    t2 = pool.tile([128, D], BF, tag="ln_t2", bufs=2, name=f"lnt2{n}")
    ew.tensor_tensor(t2[:], t[:], g[:], OP.mult)
    ew.tensor_tensor(out[:], t2[:], be[:], OP.add)


def prep_inputs(x, Wq, bq, Wk, bk, Wv, bv, Wo, bo, W1, b1, W2, b2,
                g1, be1, g2, be2):
    """Host-side prep: per-core inputs, weights pre-cast to fp8e4/bf16.

    Weight fp8 layouts are [*, 128, n] with the 128 SBUF partitions
    contiguous-major so each tile is one dense DMA; columns are dc-major
    so (dc, dc+1) pairs sit adjacent for DoubleRow APs. Wk/Wq/Wv/W1 are
    pre-scaled by WS (W2 by WS2) to sit in e4m3 normal range; bo is
    folded into the xn residual input; bv is pre-scaled by WS to match
    V' = WS*V.
    """
    import ml_dtypes
    f = np.float32
    bf = ml_dtypes.bfloat16
    f8 = ml_dtypes.float8_e4m3

    def _qdc(w, ncol, scale):  # [D_in, ncols] -> [ncols/ncol, 128, 8*ncol]
        # element (blk, q, dc*ncol+c) = scale * w[dc*128+q, blk*ncol+c]
        nblk = w.shape[1] // ncol
        return np.ascontiguousarray(
            (np.asarray(w, f) * scale).reshape(8, 128, nblk, ncol)
            .transpose(2, 1, 0, 3).reshape(nblk, 128, 8 * ncol)).astype(f8)

    wq_flat = np.asarray(Wq, f).transpose(1, 0, 2).reshape(D, D)
    wk_flat = np.asarray(Wk, f).transpose(1, 0, 2).reshape(D, D)
    wv_flat = np.asarray(Wv, f).transpose(1, 0, 2).reshape(D, D)
    common = {
        "wkq": np.ascontiguousarray(np.concatenate(
            [_qdc(wk_flat, 128, WS), _qdc(wq_flat, 128, WS)], axis=2)),
        "wv": _qdc(wv_flat, 512, WS),
        "wo": np.ascontiguousarray(
            np.asarray(Wo, f).reshape(2, 4, 128, D).transpose(0, 2, 1, 3)
            .reshape(2, 128, 4 * D)).astype(bf),
        "w1": _qdc(np.asarray(W1, f), 128, WS),
        # [128, 32*1024]: w2[q, f*1024 + c] = WS2 * W2[f*128+q, c]
        "w2": np.ascontiguousarray(
            (np.asarray(W2, f) * WS2).reshape(32, 128, D)
            .transpose(1, 0, 2).reshape(128, 32 * D)).astype(f8),
        "bq": np.asarray(bq, f).reshape(D, 1),
        "bk": np.asarray(bk, f).reshape(D, 1),
        "bvr": (np.asarray(bv, f) * WS).reshape(1, D).astype(bf),
        "b1": np.asarray(b1, f).reshape(DFF, 1),
        "b2r": np.asarray(b2, f).reshape(1, D).astype(bf),
        "g1": np.asarray(g1, f).reshape(1, D).astype(bf),
        "be1": np.asarray(be1, f).reshape(1, D).astype(bf),
        "g2": np.asarray(g2, f).reshape(1, D).astype(bf),
        "be2": np.asarray(be2, f).reshape(1, D).astype(bf),
        "ident": np.eye(128, dtype=f).astype(bf),
    }
    xf = np.asarray(x, f)
    bo_f = np.asarray(bo, f).reshape(1, D)
    # xbT piece-major: [p, piece, dc, col] = x[b].T[dc*128+p, piece*512+col]
    xbT = []
    for b in range(B):
        xt = np.ascontiguousarray(xf[b].T).astype(f8).astype(f)  # [D, FB]
        pm = (xt.reshape(8, 128, 4, 512).transpose(1, 2, 0, 3)
              .reshape(128, 4 * 8 * 512))
        xbT.append(np.ascontiguousarray(pm).astype(f8))
    in_maps = []
    for c in range(NCORES):
        b, j = divmod(c, 4)
        m = dict(common)
        m["xbT"] = xbT[b]
        own = xf[b, j * RPC:(j + 1) * RPC, :]
        xo = np.ascontiguousarray(own.T).astype(f8).astype(f)  # [D, RPC]
        m["xoT"] = np.ascontiguousarray(
            xo.reshape(8, 128, 512).transpose(1, 0, 2)
            .reshape(128, 8 * 512)).astype(f8)
        m["xn"] = np.ascontiguousarray(own + bo_f).astype(bf)
        in_maps.append(m)
    return in_maps


_NC_CACHE = {}
LAST_EXEC_NS = None
LAST_TRACE_PATH = None
LAST_PROFILE_JSON = None


def kernel(**inputs) -> np.ndarray:
    global LAST_EXEC_NS, LAST_TRACE_PATH, LAST_PROFILE_JSON
    if "main" not in _NC_CACHE:
        _NC_CACHE["main"] = build_nc()
    nc = _NC_CACHE["main"]
    in_maps = prep_inputs(**inputs)
    res = run_bass_kernel_spmd(nc, in_maps, core_ids=list(range(NCORES)))
    LAST_EXEC_NS = getattr(res, "exec_time_ns", None)
    LAST_PROFILE_JSON = getattr(res, "profile_json", None)
    it = getattr(res, "instructions_and_trace", None)
    LAST_TRACE_PATH = it[1] if it else None
    out = np.empty((B, S, D), np.float32)
    for c in range(NCORES):
        b, j = divmod(c, 4)
        out[b, j * RPC:(j + 1) * RPC, :] = res.results[c]["out"]
    return out


# revision 21
# speedup vs baseline: 1.1749x; 1.1749x over previous
"""Trainium2 Bass kernel for nn_Encoder (dense transformer encoder layer).

Sharding: 8 NeuronCores, sequence-parallel, zero collectives. B*S = 4096
rows -> 512 rows per core; cores 0-3 handle batch 0, cores 4-7 batch 1.
The full batch's x (transposed, fp8e4) is replicated to every core as an
input, so each core computes K^T and V for the WHOLE batch locally (no
K/V AllGather), then runs attention for its own 512 query rows over all
16 heads, plus Wo / LN1 / FFN / LN2 locally.

FP8 (e4m3) DoubleRow matmuls for the big-contraction GEMMs: K/Q/V
projections, P@V, FFN1, FFN2 (2 fp8 values per PE cell -> 2x contraction
per cycle). Weights are pre-scaled x32 (W2 x64) host-side so they sit in
e4m3 normal range; the inverse scales are folded into the PSUM
evacuations (tensor_scalar mult+add / activation scale). Scores matmul
stays bf16 (output-rate-bound at K=64, fp8 gains nothing); Wo stays
bf16. Residual path keeps bf16 precision. PSUM always fp32.

x is shipped pre-transposed in a piece-major layout [128, 4, 8, 512]
(p, key-piece, dc, col) so each of the 4 input DMAs is one contiguous
4KB line per partition AND matmul APs can slice (dc, dc+1) pairs for
DoubleRow directly. First projection matmul starts ~2us in.

Schedule: every projection chunk (K/Q/V) carries a (pair, unit)
deadline; chunks are emitted just-in-time between attention units.

Dataflow per core:
  xTs [128, 4, 8, 512] fp8 (host-transposed full-batch input)
  kT[p] [128, 2048] bf16 = (Wk_p'.T @ xT)/32 + bk  (full batch keys)
  vq[half]: V' = 32*V + 32*bv columns (fp8) for 4 pairs, keys on
            partition, ones column at col 64, pad cols 65:80 zero
  qT[p] [128, 512] bf16 from xoT fp8 (own rows)
  S^T [128 keys, 512 q] = kts.T @ qT  (2 heads packed at rows 0/64)
  P = exp(0.125 * S^T) -> fp8, one ACT op per 2 key tiles
  O^T[80,512] += DoubleRow([V'|1|0].T @ P)  (psum row 64 = denominator)
  oT = O'^T * bcast((1/32)*(1/den));  x1 = sum_dc oT_chunk.T @ Wo_rows
       + (x + bo)  [bo folded host-side into xn]
  LN1 -> x1n bf16 -> x1nT fp8;  hT = relu((W1'.T @ x1nT)/32 + b1) fp8
  ffn = (sum_f DoubleRow(hT_chunk.T @ W2'_rows))/64; x2 = ffn + x1n + b2
  LN2 -> out [512, 1024] f32
"""

from collections import deque

import numpy as np

import concourse.bass as bass
import concourse.mybir as mybir
from concourse import bacc
from concourse.tile import TileContext
from concourse.bass_utils import run_bass_kernel_spmd

F32 = mybir.dt.float32
BF = mybir.dt.bfloat16
F8 = mybir.dt.float8e4
AF = mybir.ActivationFunctionType
OP = mybir.AluOpType
DR = mybir.MatmulPerfMode.DoubleRow

B, S, D = 2, 2048, 1024
H, DK, DFF = 16, 64, 4096
NCORES = 8
RPC = S * B // NCORES          # 512 own rows per core
FB = S                         # 2048 full-batch rows
NP = H // 2                    # 8 head pairs
NKT = FB // 128                # 16 key tiles
VC = 80                        # padded V column block (64 V + 1 ones + 15 pad)
WS = 32.0                      # weight pre-scale for Wk/Wq/Wv/W1
WS2 = 64.0                     # weight pre-scale for W2

_TCNT = [0]


def _mk(pool, shape, dt, tag, bufs=None):
    _TCNT[0] += 1
    kw = {} if bufs is None else {"bufs": bufs}
    return pool.tile(shape, dt, tag=tag, name=f"t{_TCNT[0]}_{tag}", **kw)


def build_nc():
    nc = bacc.Bacc(num_devices=NCORES)

    ins = dict(
        xbT=nc.dram_tensor("xbT", [128, 4 * 8 * 512], F8, kind="ExternalInput"),
        xoT=nc.dram_tensor("xoT", [128, 8 * 512], F8, kind="ExternalInput"),
        xn=nc.dram_tensor("xn", [RPC, D], BF, kind="ExternalInput"),
        wkq=nc.dram_tensor("wkq", [NP, 128, 2048], F8, kind="ExternalInput"),
        wv=nc.dram_tensor("wv", [2, 128, 4096], F8, kind="ExternalInput"),
        wo=nc.dram_tensor("wo", [2, 128, 4096], BF, kind="ExternalInput"),
        w1=nc.dram_tensor("w1", [32, 128, 1024], F8, kind="ExternalInput"),
        w2=nc.dram_tensor("w2", [128, 32 * 1024], F8, kind="ExternalInput"),
        bq=nc.dram_tensor("bq", [D, 1], F32, kind="ExternalInput"),
        bk=nc.dram_tensor("bk", [D, 1], F32, kind="ExternalInput"),
        bvr=nc.dram_tensor("bvr", [1, D], BF, kind="ExternalInput"),
        b1=nc.dram_tensor("b1", [DFF, 1], F32, kind="ExternalInput"),
        b2r=nc.dram_tensor("b2r", [1, D], BF, kind="ExternalInput"),
        g1=nc.dram_tensor("g1", [1, D], BF, kind="ExternalInput"),
        be1=nc.dram_tensor("be1", [1, D], BF, kind="ExternalInput"),
        g2=nc.dram_tensor("g2", [1, D], BF, kind="ExternalInput"),
        be2=nc.dram_tensor("be2", [1, D], BF, kind="ExternalInput"),
        ident=nc.dram_tensor("ident", [128, 128], BF, kind="ExternalInput"),
        out=nc.dram_tensor("out", [RPC, D], F32, kind="ExternalOutput"),
    )

    with TileContext(nc) as tc:
        _body(nc, tc, ins)

    nc.finalize()
    return nc


def _body(nc, tc, ins):
    with (
        tc.tile_pool(name="outer", bufs=1) as po,
        tc.tile_pool(name="psum", bufs=1, space="PSUM") as pp,
    ):
        # ---- constants (bvr first: needed by the first V evacuation) ----
        bvr = _mk(po, [128, D], BF, "bvr")
        nc.gpsimd.dma_start(out=bvr[:], in_=ins["bvr"].broadcast_to([128, D]))
        ident = _mk(po, [128, 128], BF, "ident")
        nc.gpsimd.dma_start(out=ident[:], in_=ins["ident"][:])
        bias = {}
        for nm, n in (("bq", 8), ("bk", 8), ("b1", 32)):
            t = _mk(po, [128, n], F32, "b_" + nm)
            nc.gpsimd.dma_start(out=t[:],
                                in_=ins[nm].rearrange("(i p) o -> p (i o)", p=128))
            bias[nm] = t
        b2r = _mk(po, [128, D], BF, "b2r")
        nc.gpsimd.dma_start(out=b2r[:], in_=ins["b2r"].broadcast_to([128, D]))
        lnw = {}
        for nm in ("g1", "be1", "g2", "be2"):
            t = _mk(po, [128, D], BF, "ln_" + nm)
            nc.gpsimd.dma_start(out=t[:], in_=ins[nm].broadcast_to([128, D]))
            lnw[nm] = t
        eps = _mk(po, [128, 1], F32, "eps")
        nc.vector.memset(eps[:], 1e-5)
        lnw["eps"] = eps
        # broadcast source for 1/den: value 1/WS folds the V' = WS*V scale
        ones_f = _mk(po, [128, 64], F32, "ones_f")
        nc.vector.memset(ones_f[:], 1.0 / WS)
        # xn is DMA'd late (inside the weave) so it doesn't block the
        # startup-critical xoT/wkq transfers.
        xn = [_mk(po, [128, D], BF, f"xn{r}") for r in range(4)]

        # persistent post-phase activations
        x1n = [_mk(po, [128, D], BF, f"x1n{r}") for r in range(4)]
        x1nb = [_mk(po, [128, D], BF, f"x1nb{r}") for r in range(4)]
        x1nT = _mk(po, [128, 8 * RPC], F8, "x1nT")
        w2t = _mk(po, [128, 32 * 1024], F8, "w2t")

        with tc.tile_pool(name="attn", bufs=1) as pa:
            _attn_phase(nc, tc, ins, po, pa, pp, ident, bias, bvr, b2r,
                        xn, x1n, x1nb, x1nT, w2t, lnw, ones_f)

        with tc.tile_pool(name="post", bufs=1) as pf:
            _post_phase(nc, tc, ins, pf, pp, bias, lnw, x1n, x1nb, x1nT, w2t)


def _attn_phase(nc, tc, ins, po, pa, pp, ident, bias, bvr, b2r,
                xn, x1n, x1nb, x1nT, w2t, lnw, ones_f):
    oT = [_mk(pa, [128, RPC], BF, f"oT{p}") for p in range(NP)]
    kTs, qTs, vqs = {}, {}, {}

    with tc.tile_pool(name="proj", bufs=1) as px:
        # x^T in piece-major layout: [p, piece, dc, col] — each piece DMA is
        # one contiguous 4KB line per partition; matmul APs slice (dc,dc+1)
        # pairs for DoubleRow.
        # startup-critical transfers: pieces 0/2 on the sync queue now;
        # wkq0, xoT, pieces 1/3 go on the scalar queue inside dma_w(0) so
        # wkq0 is first in that queue's order.
        xTs = _mk(px, [128, 4 * 8 * 512], F8, "xTs")
        xTv = xTs[:].rearrange("p (k d c) -> p k d c", k=4, d=8)
        xbTv = ins["xbT"].rearrange("p (k d c) -> p k d c", k=4, d=8)
        xoTs = _mk(px, [128, 8 * 512], F8, "xoTs")
        xoTv = xoTs[:].rearrange("p (d c) -> p d c", d=8)
        nc.sync.dma_start(out=xTv[:, 0], in_=xbTv[:, 0])
        nc.sync.dma_start(out=xTv[:, 2], in_=xbTv[:, 2])

        def proj_items(p):
            """(pair, unit, thunk) items, sorted by deadline."""
            st = {}

            def dma_w():
                wkqt = _mk(px, [128, 2048], F8, "wkq", bufs=2)
                nc.scalar.dma_start(out=wkqt[:], in_=ins["wkq"][p])
                if p == 0:
                    nc.scalar.dma_start(
                        out=xoTv[:],
                        in_=ins["xoT"].rearrange("p (d c) -> p d c", d=8))
                    nc.scalar.dma_start(out=xTv[:, 1], in_=xbTv[:, 1])
                    nc.scalar.dma_start(out=xTv[:, 3], in_=xbTv[:, 3])
                kTs[p] = _mk(pa, [128, FB], BF, "kT", bufs=3)
                qTs[p] = _mk(pa, [128, RPC], BF, "qT", bufs=3)
                st["wkt"] = wkqt[:, 0:1024].rearrange("p (d c) -> p d c", d=8)
                st["wqt"] = wkqt[:, 1024:2048].rearrange("p (d c) -> p d c", d=8)

            def k_chunk(c):
                ps = _mk(pp, [128, 512], F32, "ps_p", bufs=2)
                for dc in range(0, 8, 2):
                    nc.tensor.matmul(ps[:], st["wkt"][:, dc:dc + 2, :],
                                     xTv[:, c, dc:dc + 2, :],
                                     start=(dc == 0), stop=(dc == 6),
                                     perf_mode=DR, skip_group_check=True)
                nc.vector.tensor_scalar(kTs[p][:, c * 512:(c + 1) * 512],
                                        ps[:], 1.0 / WS, bias["bk"][:, p:p + 1],
                                        OP.mult, OP.add)

            def q_chunk():
                ps = _mk(pp, [128, 512], F32, "ps_p", bufs=2)
                for dc in range(0, 8, 2):
                    nc.tensor.matmul(ps[:], st["wqt"][:, dc:dc + 2, :],
                                     xoTv[:, dc:dc + 2, :],
                                     start=(dc == 0), stop=(dc == 6),
                                     perf_mode=DR, skip_group_check=True)
                nc.vector.tensor_scalar(qTs[p][:], ps[:], 1.0 / WS,
                                        bias["bq"][:, p:p + 1],
                                        OP.mult, OP.add)

            items = [(p, 0, dma_w), (p, 0, lambda: k_chunk(0)), (p, 0, q_chunk)]
            for c in range(1, 4):
                items.append((p, 2 * c, lambda c=c: k_chunk(c)))

            if p % 4 == 0:
                hf = p // 4

                def dma_v():
                    wvt = _mk(px, [128, 4096], F8, "wvt", bufs=1)
                    nc.sync.dma_start(out=wvt[:], in_=ins["wv"][hf])
                    vq = _mk(pa, [128, 4 * 2 * NKT * VC], F8, "vq", bufs=2)
                    vqs[hf] = vq
                    vqv = vq[:].rearrange("k (i h t c) -> k i h t c",
                                          i=4, h=2, c=VC)
                    nc.vector.memset(vqv[:, :, :, :, 64:VC], 0.0)
                    nc.vector.memset(vqv[:, :, :, :, 64:65], 1.0)
                    st["wvt"] = wvt[:].rearrange("p (d c) -> p d c", d=8)

                def v_chunk(kt2):
                    vqv = vqs[hf][:].rearrange("k (i h t c) -> k i h t c",
                                               i=4, h=2, c=VC)
                    bvs = bvr[:, hf * 512:(hf + 1) * 512].rearrange(
                        "k (i h c) -> k i h c", i=4, c=64)
                    ps = _mk(pp, [128, 512], F32, "ps_p", bufs=2)
                    for dc in range(0, 8, 2):
                        nc.tensor.matmul(
                            ps[:],
                            xTv[:, kt2 // 4, dc:dc + 2,
                                (kt2 % 4) * 128:(kt2 % 4 + 1) * 128],
                            st["wvt"][:, dc:dc + 2, :],
                            start=(dc == 0), stop=(dc == 6),
                            perf_mode=DR, skip_group_check=True)
                    nc.vector.tensor_tensor(
                        vqv[:, :, :, kt2, 0:64],
                        ps[:].rearrange("k (i h c) -> k i h c", i=4, c=64),
                        bvs, OP.add)

                items.append((p, 0, dma_v))
                for kt2 in range(NKT):
                    items.append((p, kt2 // 2, lambda kt2=kt2: v_chunk(kt2)))

            items.sort(key=lambda it: (it[0], it[1]))
            return items

        def attn_unit(p, u, pso):
            kt, qt, vq = kTs[p], qTs[p], vqs[p // 4]
            pl = p % 4
            vqv = vq[:].rearrange("k (i h t c) -> k i h t c", i=4, h=2, c=VC)
            pss = [_mk(pp, [128, 1024], F32, "ps_s", bufs=2) for _ in range(2)]
            for i in range(2):
                ktile = 2 * u + i
                for hh in range(2):
                    nc.tensor.matmul(
                        pss[hh][:, i * 512:(i + 1) * 512],
                        kt[hh * 64:(hh + 1) * 64,
                           ktile * 128:(ktile + 1) * 128],
                        qt[hh * 64:(hh + 1) * 64, :],
                        start=True, stop=True, skip_group_check=True)
            for hh in range(2):
                pt = _mk(pa, [128, 1024], F8, "pt", bufs=2)
                nc.scalar.activation(pt[:], pss[hh][:], AF.Exp,
                                     bias=0.0, scale=0.125)
                nc.tensor.matmul(
                    pso[hh][0:VC, :],
                    vqv[:, pl, hh, 2 * u:2 * u + 2, :],
                    pt[:].rearrange("k (i q) -> k i q", i=2),
                    start=(u == 0), stop=(u == NKT // 2 - 1),
                    perf_mode=DR, skip_group_check=True)

        def attn_stash(p, pso):
            """Copy raw O and den off PSUM at pair end (frees the banks);
            normalization is deferred into the next pair's stream."""
            st = []
            den = _mk(pa, [128, 512], BF, "dens", bufs=2)
            for hh in range(2):
                o_raw = _mk(pa, [128, 512], BF, "o_raw", bufs=2)
                nc.vector.tensor_copy(o_raw[0:64, :], pso[hh][0:64, :])
                nc.vector.tensor_copy(den[64 * hh:64 * hh + 1, :],
                                      pso[hh][64:65, :])
                st.append(o_raw)
            st.append(den)
            return st

        def attn_normalize(p, st):
            den = st[2]
            rden = _mk(pa, [128, 512], F32, "rden", bufs=1)
            for hh in range(2):
                nc.vector.reciprocal(rden[64 * hh:64 * hh + 1, :],
                                     den[64 * hh:64 * hh + 1, :])
            for hh in range(2):
                o_raw = st[hh]
                ps_b = _mk(pp, [128, 512], F32, "ps_p", bufs=2)
                nc.tensor.matmul(ps_b[0:64, :],
                                 ones_f[64 * hh:64 * hh + 1, 0:64],
                                 rden[64 * hh:64 * hh + 1, :],
                                 start=True, stop=True,
                                 skip_group_check=True)
                rb = _mk(pa, [128, 512], F32, "rb", bufs=1)
                nc.vector.tensor_copy(rb[0:64, :], ps_b[0:64, :])
                # bv already folded into V' (softmax rows sum to 1)
                if hh == 0:
                    nc.gpsimd.tensor_tensor(oT[p][0:64, :], o_raw[0:64, :],
                                            rb[0:64, :], OP.mult)
                else:
                    stage = _mk(pa, [128, 512], BF, "stage", bufs=1)
                    nc.gpsimd.tensor_tensor(stage[0:64, :], o_raw[0:64, :],
                                            rb[0:64, :], OP.mult)
                    nc.gpsimd.dma_start(out=oT[p][64:128, :],
                                        in_=stage[0:64, :])

        # ---- deadline-scheduled weave (proj runs up to 2 pairs ahead;
        # kT/qT tag bufs=3 require never enqueueing more than p+2) ----
        pending = deque()
        for pre in range(min(2, NP)):
            pending.extend(proj_items(pre))

        def drain(n):
            for _ in range(min(n, len(pending))):
                pending.popleft()[2]()

        def drain_due(p, u):
            while pending and (pending[0][0], pending[0][1]) <= (p, u):
                pending.popleft()[2]()

        wo_big = [_mk(pa, [128, 4096], BF, f"wot{g}") for g in range(2)]

        def wo_slice(dc, hf):
            return wo_big[dc // 4][:, (dc % 4) * 1024 + hf * 512:
                                   (dc % 4) * 1024 + (hf + 1) * 512]

        wo_parts = {}

        def wo_part(r):
            ps = [_mk(pp, [128, 512], F32, "ps_p", bufs=2) for _ in range(2)]
            for dc in range(6):
                for hf in range(2):
                    nc.tensor.matmul(ps[hf][:],
                                     oT[dc][:, r * 128:(r + 1) * 128],
                                     wo_slice(dc, hf),
                                     start=(dc == 0), stop=(dc == 5),
                                     skip_group_check=True)
            parts = []
            for hf in range(2):
                part = _mk(pa, [128, 512], BF, "wopart", bufs=8)
                nc.vector.tensor_copy(part[:], ps[hf][:])
                parts.append(part)
            wo_parts[r] = parts

        stash = None
        for p in range(NP):
            if p + 2 < NP:
                pending.extend(proj_items(p + 2))
            pso = [_mk(pp, [128, 512], F32, "ps_o", bufs=2) for _ in range(2)]
            for u in range(NKT // 2):
                drain_due(p, u)
                attn_unit(p, u, pso)
                if u == 3 and stash is not None:
                    attn_normalize(p - 1, stash)
                # late weight loads ride the scalar/gpsimd queues whose
                # instruction streams are gated by attention progress (EXP /
                # stage copies), so they can't crowd the startup window.
                if p == 2 and u == 0:
                    for r in range(4):
                        nc.gpsimd.dma_start(
                            out=xn[r][:],
                            in_=ins["xn"][r * 128:(r + 1) * 128, :])
                if p == 4 and u == 0:
                    for g in range(2):
                        nc.scalar.dma_start(out=wo_big[g][:], in_=ins["wo"][g])
                if p == 4 and u == 4:
                    nc.scalar.dma_start(out=w2t[:], in_=ins["w2"][:])
                if (p, u) in ((6, 4), (6, 6), (7, 1), (7, 3)):
                    wo_part({(6, 4): 0, (6, 6): 1, (7, 1): 2, (7, 3): 3}[(p, u)])
                drain(2)
            stash = attn_stash(p, pso)
        attn_normalize(NP - 1, stash)
        drain(len(pending))

    # ---- Wo combine: add the dc 6..7 contributions, then LN1 ----
    pst = [_mk(pp, [128, 2048], BF, "ps_s", bufs=2) for _ in range(2)]
    x1nTv = x1nT[:].rearrange("p (d c) -> p d c", d=8)
    for r in range(4):
        ps = [_mk(pp, [128, 512], F32, "ps_p", bufs=2) for _ in range(2)]
        for dc in range(6, 8):
            for hf in range(2):
                nc.tensor.matmul(ps[hf][:],
                                 oT[dc][:, r * 128:(r + 1) * 128],
                                 wo_slice(dc, hf),
                                 start=(dc == 6), stop=(dc == 7),
                                 skip_group_check=True)
        x1 = _mk(po, [128, D], BF, "x1", bufs=2)
        for hf in range(2):
            t = _mk(po, [128, 512], BF, "x1t", bufs=2)
            nc.vector.tensor_tensor(t[:], ps[hf][:], wo_parts[r][hf][:],
                                    OP.add)
            nc.vector.tensor_tensor(x1[:, hf * 512:(hf + 1) * 512],
                                    t[:], xn[r][:, hf * 512:(hf + 1) * 512],
                                    OP.add)
        _layernorm(nc, po, x1n[r], x1, lnw["g1"], lnw["be1"], lnw["eps"])
        nc.vector.tensor_tensor(x1nb[r][:], x1n[r][:], b2r[:], OP.add)
        for dc in range(8):
            nc.tensor.transpose(
                pst[dc // 4][:, (dc % 4) * 512 + r * 128:
                             (dc % 4) * 512 + (r + 1) * 128],
                x1n[r][:, dc * 128:(dc + 1) * 128], ident[:])
    for dc in range(8):
        nc.scalar.activation(x1nTv[:, dc, :],
                             pst[dc // 4][:, (dc % 4) * 512:
                                          (dc % 4 + 1) * 512], AF.Copy)


def _post_phase(nc, tc, ins, pf, pp, bias, lnw, x1n, x1nb, x1nT, w2t):
    # ---- FFN1: hT[f] = relu((W1'.T @ x1nT)/WS + b1), fp8 out ----
    hT = _mk(pf, [128, 32 * RPC], F8, "hT")
    hTv = hT[:].rearrange("p (f c) -> p f c", f=32)
    x1nTv = x1nT[:].rearrange("p (d c) -> p d c", d=8)
    w2v = w2t[:].rearrange("p (f c) -> p f c", f=32)
    for f in range(32):
        w1t = _mk(pf, [128, 1024], F8, "w1t", bufs=4)
        nc.sync.dma_start(out=w1t[:], in_=ins["w1"][f])
        w1v = w1t[:].rearrange("p (d c) -> p d c", d=8)
        ps = _mk(pp, [128, 512], F32, "ps_p", bufs=2)
        for dc in range(0, 8, 2):
            nc.tensor.matmul(ps[:], w1v[:, dc:dc + 2, :],
                             x1nTv[:, dc:dc + 2, :],
                             start=(dc == 0), stop=(dc == 6),
                             perf_mode=DR, skip_group_check=True)
        nc.scalar.activation(hTv[:, f, :], ps[:], AF.Relu,
                             bias=bias["b1"][:, f:f + 1], scale=1.0 / WS)

    # ---- FFN2 (DoubleRow) + residual + LN2 + out ----
    for r in range(4):
        ps = _mk(pp, [128, 1024], F32, "ps_s", bufs=2)
        for f in range(0, 32, 2):
            for hf in range(2):
                nc.tensor.matmul(ps[:, hf * 512:(hf + 1) * 512],
                                 hTv[:, f:f + 2, r * 128:(r + 1) * 128],
                                 w2v[:, f:f + 2, hf * 512:(hf + 1) * 512],
                                 start=(f == 0), stop=(f == 30),
                                 perf_mode=DR, skip_group_check=True)
        x2 = _mk(pf, [128, D], BF, "x2", bufs=2)
        for hf in range(2):
            sc = _mk(pf, [128, 512], BF, "ffnsc", bufs=2)
            nc.scalar.activation(sc[:], ps[:, hf * 512:(hf + 1) * 512],
                                 AF.Copy, bias=0.0, scale=1.0 / WS2)
            nc.vector.tensor_tensor(x2[:, hf * 512:(hf + 1) * 512], sc[:],
                                    x1nb[r][:, hf * 512:(hf + 1) * 512],
                                    OP.add)
        outt = _mk(pf, [128, D], F32, "outt", bufs=2)
        _layernorm(nc, pf, outt, x2, lnw["g2"], lnw["be2"], lnw["eps"])
        nc.sync.dma_start(out=ins["out"][r * 128:(r + 1) * 128, :],
                          in_=outt[:])


def _layernorm(nc, pool, out, x, g, be, eps, ew=None):
    """LN along the free dim (D=1024). x [128, 1024] bf16; out bf16/f32.

    1/std via exp(-0.5*ln(var+eps)) on the scalar engine (vector
    reciprocal has a ~2.2us fixed cost). `ew` picks the engine for the
    elementwise tail (default vector; gpsimd to offload busy regions).
    """
    if ew is None:
        ew = nc.vector
    _TCNT[0] += 1
    n = _TCNT[0]
    stats = pool.tile([128, 2, 6], F32, tag="ln_st", bufs=2, name=f"lnst{n}")
    for i in range(2):
        nc.vector.bn_stats(stats[:, i, :], x[:, i * 512:(i + 1) * 512])
    mv = pool.tile([128, 2], F32, tag="ln_mv", bufs=2, name=f"lnmv{n}")
    nc.vector.bn_aggr(mv[:], stats[:])
    lnv = pool.tile([128, 1], F32, tag="ln_sd", bufs=2, name=f"lnsd{n}")
    nc.scalar.activation(lnv[:], mv[:, 1:2], AF.Ln, bias=eps, scale=1.0)
    rstd = pool.tile([128, 1], F32, tag="ln_rs", bufs=2, name=f"lnrs{n}")
    nc.scalar.activation(rstd[:], lnv[:], AF.Exp, bias=0.0, scale=-0.5)
    t = pool.tile([128, D], BF, tag="ln_t", bufs=2, name=f"lnt{n}")
    ew.tensor_scalar(t[:], x[:], mv[:, 0:1], rstd[:],
                     OP.subtract, OP.mult)
    t2 = pool.tile([128, D], BF, tag="ln_t2", bufs=2, name=f"lnt2{n}")
    ew.tensor_tensor(t2[:], t[:], g[:], OP.mult)
    ew.tensor_tensor(out[:], t2[:], be[:], OP.add)


def prep_inputs(x, Wq, bq, Wk, bk, Wv, bv, Wo, bo, W1, b1, W2, b2,
                g1, be1, g2, be2):
    """Host-side prep: per-core inputs, weights pre-cast to fp8e4/bf16.

    Weight fp8 layouts are [*, 128, n] with the 128 SBUF partitions
    contiguous-major so each tile is one dense DMA; columns are dc-major
    so (dc, dc+1) pairs sit adjacent for DoubleRow APs. Wk/Wq/Wv/W1 are
    pre-scaled by WS (W2 by WS2) to sit in e4m3 normal range; bo is
    folded into the xn residual input; bv is pre-scaled by WS to match
    V' = WS*V.
    """
    import ml_dtypes
    f = np.float32
    bf = ml_dtypes.bfloat16
    f8 = ml_dtypes.float8_e4m3

    def _qdc(w, ncol, scale):  # [D_in, ncols] -> [ncols/ncol, 128, 8*ncol]
        # element (blk, q, dc*ncol+c) = scale * w[dc*128+q, blk*ncol+c]
        nblk = w.shape[1] // ncol
        return np.ascontiguousarray(
            (np.asarray(w, f) * scale).reshape(8, 128, nblk, ncol)
            .transpose(2, 1, 0, 3).reshape(nblk, 128, 8 * ncol)).astype(f8)

    wq_flat = np.asarray(Wq, f).transpose(1, 0, 2).reshape(D, D)
    wk_flat = np.asarray(Wk, f).transpose(1, 0, 2).reshape(D, D)
    wv_flat = np.asarray(Wv, f).transpose(1, 0, 2).reshape(D, D)
    common = {
        "wkq": np.ascontiguousarray(np.concatenate(
            [_qdc(wk_flat, 128, WS), _qdc(wq_flat, 128, WS)], axis=2)),
        "wv": _qdc(wv_flat, 512, WS),
        "wo": np.ascontiguousarray(
            np.asarray(Wo, f).reshape(2, 4, 128, D).transpose(0, 2, 1, 3)
            .reshape(2, 128, 4 * D)).astype(bf),
        "w1": _qdc(np.asarray(W1, f), 128, WS),
        # [128, 32*1024]: w2[q, f*1024 + c] = WS2 * W2[f*128+q, c]
        "w2": np.ascontiguousarray(
            (np.asarray(W2, f) * WS2).reshape(32, 128, D)
            .transpose(1, 0, 2).reshape(128, 32 * D)).astype(f8),
        "bq": np.asarray(bq, f).reshape(D, 1),
        "bk": np.asarray(bk, f).reshape(D, 1),
        "bvr": (np.asarray(bv, f) * WS).reshape(1, D).astype(bf),
        "b1": np.asarray(b1, f).reshape(DFF, 1),
        "b2r": np.asarray(b2, f).reshape(1, D).astype(bf),
        "g1": np.asarray(g1, f).reshape(1, D).astype(bf),
        "be1": np.asarray(be1, f).reshape(1, D).astype(bf),
        "g2": np.asarray(g2, f).reshape(1, D).astype(bf),
        "be2": np.asarray(be2, f).reshape(1, D).astype(bf),
        "ident": np.eye(128, dtype=f).astype(bf),
    }
    xf = np.asarray(x, f)
    bo_f = np.asarray(bo, f).reshape(1, D)
    # xbT piece-major: [p, piece, dc, col] = x[b].T[dc*128+p, piece*512+col]
    xbT = []
    for b in range(B):
        xt = np.ascontiguousarray(xf[b].T).astype(f8).astype(f)  # [D, FB]
        pm = (xt.reshape(8, 128, 4, 512).transpose(1, 2, 0, 3)
              .reshape(128, 4 * 8 * 512))
        xbT.append(np.ascontiguousarray(pm).astype(f8))
    in_maps = []
    for c in range(NCORES):
        b, j = divmod(c, 4)
        m = dict(common)
        m["xbT"] = xbT[b]
        own = xf[b, j * RPC:(j + 1) * RPC, :]
        xo = np.ascontiguousarray(own.T).astype(f8).astype(f)  # [D, RPC]
        m["xoT"] = np.ascontiguousarray(
            xo.reshape(8, 128, 512).transpose(1, 0, 2)
            .reshape(128, 8 * 512)).astype(f8)
        m["xn"] = np.ascontiguousarray(own + bo_f).astype(bf)
        in_maps.append(m)
    return in_maps


_NC_CACHE = {}
LAST_EXEC_NS = None
LAST_TRACE_PATH = None
LAST_PROFILE_JSON = None


def kernel(**inputs) -> np.ndarray:
    global LAST_EXEC_NS, LAST_TRACE_PATH, LAST_PROFILE_JSON
    if "main" not in _NC_CACHE:
        _NC_CACHE["main"] = build_nc()
    nc = _NC_CACHE["main"]
    in_maps = prep_inputs(**inputs)
    res = run_bass_kernel_spmd(nc, in_maps, core_ids=list(range(NCORES)))
    LAST_EXEC_NS = getattr(res, "exec_time_ns", None)
    LAST_PROFILE_JSON = getattr(res, "profile_json", None)
    it = getattr(res, "instructions_and_trace", None)
    LAST_TRACE_PATH = it[1] if it else None
    out = np.empty((B, S, D), np.float32)
    for c in range(NCORES):
        b, j = divmod(c, 4)
        out[b, j * RPC:(j + 1) * RPC, :] = res.results[c]["out"]
    return out
